# revision 12
# baseline (speedup 1.0000x reference)
"""Trainium2 Bass kernel for CCHead (criss-cross attention head).

Self-contained: kernel(**inputs) takes the full unsharded inputs
(x[8, 2048, 64, 64] + weights), shards batch across 8 NeuronCores
(1 image per core, all params replicated), and returns the full
output [8, 104, 64, 64] float32.

Design: all-bf16 matmuls (PSUM fp32), x pre-padded on host so every conv
window is one contiguous DMA, src ping-pong entirely in SBUF (no DRAM
roundtrips between stages), convs as 4 passes of 16 rows with [128,8,64]
psum tiles, split-EH/EW attention energies (no partition-collapse DMAs),
batched softmax with DVE row-sum reduction, and early V-transpose chunk
emission so the PE stays busy during softmax.
"""
import contextlib
import numpy as np
import ml_dtypes
import concourse.bass as bass
import concourse.tile as tile
from concourse import bacc, mybir

f32 = mybir.dt.float32
bf16 = mybir.dt.bfloat16
AF = mybir.ActivationFunctionType
AX = mybir.AxisListType
BF_NP = ml_dtypes.bfloat16

S = 65
NR = 67
FLAT = NR * S + 2          # 4357 (src tiles: lead pad + 67 padded rows + slack)
IMG0 = 1 + S               # flat offset of image row 0, col 0
XFLAT = 74 * S + 2         # padded x in DRAM
WLEN = 18 * S + 2          # conv window: 18 padded row slots + slack
X_DEV_SHAPE = (16, 128, XFLAT)
PASS_R0 = [0, 16, 32, 48]
QK_TILES = [(i * 512, 512) for i in range(8)] + [(4096, 64)]


def pad_x_host(x_core):
    """[2048, 64, 64] f32 -> [16, 128, XFLAT] bf16 padded flat."""
    xb = x_core.reshape(16, 128, 64, 64).astype(BF_NP)
    arr = np.zeros((16, 128, XFLAT), BF_NP)
    arr[:, :, 1:1 + NR * S].reshape(16, 128, NR, S)[:, :, 1:65, 0:64] = xb
    return arr


def host_prep(inputs):
    f = np.float32

    def fold(w, g, b, m, v):
        s = (g / np.sqrt(v + 1e-5)).astype(f)
        return (w * s[:, None, None, None]).astype(f), (b - m * s).astype(f)

    def wt_dev(w):  # [co, ci, 3, 3] -> [nci, 128, 9, co] bf16
        co, ci = w.shape[:2]
        return np.ascontiguousarray(
            w.reshape(co, ci, 9).transpose(1, 2, 0).reshape(
                ci // 128, 128, 9, co).astype(BF_NP))

    def t1x1(w):  # [co, ci, 1, 1] -> [nci, 128, co] bf16
        co, ci = w.shape[:2]
        return np.ascontiguousarray(
            w.reshape(co, ci).T.reshape(ci // 128, 128, co).astype(BF_NP))

    wa, ba = fold(inputs['conva_w'], inputs['conva_g'], inputs['conva_b'],
                  inputs['conva_m'], inputs['conva_v'])
    wb, bb = fold(inputs['convb_w'], inputs['convb_g'], inputs['convb_b'],
                  inputs['convb_m'], inputs['convb_v'])
    wt, bt = fold(inputs['bott_w'], inputs['bott_g'], inputs['bott_b'],
                  inputs['bott_m'], inputs['bott_v'])
    gamma = float(np.asarray(inputs['cc_gamma']).reshape(-1)[0])
    mask = np.zeros((64, 64), f)
    np.fill_diagonal(mask, -1e30)
    mask4 = np.ascontiguousarray(
        np.broadcast_to(mask[:, None, :], (64, 4, 64)).astype(f))
    dev = {
        'wa': wt_dev(wa), 'ba': ba.reshape(4, 128, 1),
        'wb': wt_dev(wb), 'bb': bb.reshape(4, 128, 1),
        'wt': wt_dev(wt), 'bt': bt.reshape(4, 128, 1),
        'wc': t1x1(inputs['cls_w']),
        'bc': inputs['cls_b'].astype(f).reshape(104, 1),
        'wq': t1x1(inputs['q_w']), 'bq': inputs['q_b'].astype(f).reshape(64, 1),
        'wk': t1x1(inputs['k_w']), 'bk': inputs['k_b'].astype(f).reshape(64, 1),
        'wv': t1x1(inputs['v_w']),
        'gvb': (gamma * inputs['v_b']).astype(f).reshape(4, 128, 1),
        'mask4': mask4,
        'ident': np.eye(64, dtype=BF_NP),
        'identf': np.eye(64, dtype=f),
    }
    return dev, gamma


INPUT_SPECS = [
    ('wa', [16, 128, 9, 512], bf16), ('ba', [4, 128, 1], f32),
    ('wb', [4, 128, 9, 512], bf16), ('bb', [4, 128, 1], f32),
    ('wt', [20, 128, 9, 512], bf16), ('bt', [4, 128, 1], f32),
    ('wc', [4, 128, 104], bf16), ('bc', [104, 1], f32),
    ('wq', [4, 128, 64], bf16), ('bq', [64, 1], f32),
    ('wk', [4, 128, 64], bf16), ('bk', [64, 1], f32),
    ('wv', [4, 128, 512], bf16),
    ('gvb', [4, 128, 1], f32),
    ('mask4', [64, 4, 64], f32),
    ('ident', [64, 64], bf16),
    ('identf', [64, 64], f32),
]


def build(gamma, n_reps=1):
    nc = bacc.Bacc("TRN2", num_devices=8)
    t = {'x': nc.dram_tensor("x", list(X_DEV_SHAPE), bf16, kind="ExternalInput")}
    for nm, shape, dt in INPUT_SPECS:
        t[nm] = nc.dram_tensor(nm, shape, dt, kind="ExternalInput")
    y = nc.dram_tensor("y", [104, 64, 64], f32, kind="ExternalOutput")
    with tile.TileContext(nc) as tc:
        _build_body(tc, t, y, gamma, n_reps)
    nc.compile()
    return nc


def _rows(flat_tile):
    """[128, FLAT] -> padded row view [128, 67, 65] (skips lead pad elem)."""
    return flat_tile[:, 1:1 + NR * S].rearrange("p (r c) -> p r c", c=S)


def _build_body(tc, t, y, gamma, n_reps):
    nc = tc.nc
    with contextlib.ExitStack() as est:
        cp = est.enter_context(tc.tile_pool(name="const", bufs=1))

        def cload(nm, shape, dt):
            tl = cp.tile(shape, dt, tag=nm, name=nm)
            nc.sync.dma_start(tl[:], t[nm][:])
            return tl

        def load_blocks(nm, n, shape, dt=f32):
            out = []
            for i in range(n):
                tl = cp.tile(shape, dt, tag=f"{nm}{i}", name=f"{nm}{i}")
                nc.sync.dma_start(tl[:], t[nm][i])
                out.append(tl)
            return out

        C = dict(nc=nc, tc=tc, t=t, y=y, gamma=gamma,
                 ident=cload('ident', [64, 64], bf16),
                 identf=cload('identf', [64, 64], f32),
                 mask4=cload('mask4', [64, 4, 64], f32),
                 bq=cload('bq', [64, 1], f32),
                 bk=cload('bk', [64, 1], f32),
                 bc=cload('bc', [104, 1], f32),
                 bias_a=load_blocks('ba', 4, [128, 1]),
                 bias_b=load_blocks('bb', 4, [128, 1]),
                 bias_t=load_blocks('bt', 4, [128, 1]),
                 gvb=load_blocks('gvb', 4, [128, 1]),
                 wq=load_blocks('wq', 4, [128, 64], bf16),
                 wk=load_blocks('wk', 4, [128, 64], bf16),
                 wv=load_blocks('wv', 4, [128, 512], bf16),
                 wc=load_blocks('wc', 4, [128, 104], bf16))

        ap = est.enter_context(tc.tile_pool(name="actp", bufs=1))
        srcA = [ap.tile([128, FLAT], bf16, tag=f"sa{i}", name=f"sa{i}")
                for i in range(4)]
        srcB = [ap.tile([128, FLAT], bf16, tag=f"sb{i}", name=f"sb{i}")
                for i in range(4)]
        for blk in srcA + srcB:
            nc.any.memset(blk[:], 0.0)
        C['srcA'], C['srcB'] = srcA, srcB

        for _ in range(n_reps):
            _network(C)


def _network(C):
    nc, tc, t = C['nc'], C['tc'], C['t']
    srcA, srcB = C['srcA'], C['srcB']
    # conva: x windows -> srcA
    with contextlib.ExitStack() as es:
        wp = es.enter_context(tc.tile_pool(name="wp", bufs=4))
        cps = es.enter_context(tc.tile_pool(name="cps", bufs=1, space="PSUM"))
        xsp = es.enter_context(tc.tile_pool(name="xsp", bufs=4))
        xg = _x_win_getter(C, xsp)
        _conv3x3(C, wp, cps, xg, 16, t['wa'], C['bias_a'], dst_sbuf=srcA)
    # CCA 1: srcA -> srcB;  CCA 2: srcB -> srcA
    _cca(C, srcA, srcB)
    _cca(C, srcB, srcA)
    # convb: srcA -> srcB
    with contextlib.ExitStack() as es:
        wp = es.enter_context(tc.tile_pool(name="wpb", bufs=3))
        cps = es.enter_context(tc.tile_pool(name="cpsb", bufs=1, space="PSUM"))
        _conv3x3(C, wp, cps, _src_getter(srcA), 4, t['wb'], C['bias_b'],
                 dst_sbuf=srcB)
    # bott: x windows (16cb) + srcB (4cb) -> ot (SBUF flat)
    with contextlib.ExitStack() as eso:
        otp = eso.enter_context(tc.tile_pool(name="otp", bufs=1))
        ot = [otp.tile([128, 64, 64], bf16, tag=f"ot{i}", name=f"ot{i}")
              for i in range(4)]
        with contextlib.ExitStack() as es:
            wp = es.enter_context(tc.tile_pool(name="wpt", bufs=4))
            cps = es.enter_context(tc.tile_pool(name="cpst", bufs=1, space="PSUM"))
            xsp = es.enter_context(tc.tile_pool(name="xspt", bufs=4))
            xg = _x_win_getter(C, xsp)
            sg = _src_getter(srcB)

            def src_get(g, cb):
                return xg(g, cb) if cb < 16 else sg(g, cb - 16)

            _conv3x3(C, wp, cps, src_get, 20, t['wt'], C['bias_t'], dst_flat=ot)
        # cls: ot (SBUF) -> y
        es = eso
        cop = es.enter_context(tc.tile_pool(name="cop", bufs=1))
        cpp = es.enter_context(tc.tile_pool(name="cpp", bufs=2, space="PSUM"))
        out_sb = cop.tile([104, 64, 64], f32)
        oflat = out_sb[:].rearrange("p r c -> p (r c)")
        for off, n in [(i * 512, 512) for i in range(8)]:
            ps = cpp.tile([104, 512], f32, tag="clsps")
            for cb in range(4):
                rhs = ot[cb][:].rearrange("p r c -> p (r c)")[:, off:off + n]
                nc.tensor.matmul(ps[:, 0:n], C['wc'][cb][:], rhs,
                                 start=(cb == 0), stop=(cb == 3))
            nc.scalar.activation(oflat[:, off:off + n], ps[:, 0:n], AF.Identity,
                                 bias=C['bc'][:], scale=1.0)
        nc.sync.dma_start(C['y'][:], out_sb[:])


def _x_win_getter(C, xsp):
    nc, t = C['nc'], C['t']
    cache = {}

    def get(g, cb):
        key = (g, cb)
        if key in cache:
            return cache[key]
        r0p = PASS_R0[g]
        xs = xsp.tile([128, WLEN], bf16, tag="xs")
        nc.sync.dma_start(xs[:], t['x'][cb][:, r0p * S:r0p * S + WLEN])
        res = (xs, lambda rr, _p=r0p: rr + 1 - _p)
        cache[key] = res
        return res

    return get


def _src_getter(src):
    def get(g, cb):
        return (src[cb], lambda rr: rr + 1)
    return get


def _conv3x3(C, wp, cps, src_getter, n_cb, w_dram, bias_sb,
             dst_sbuf=None, dst_flat=None):
    nc = C['nc']
    for g, r0p in enumerate(PASS_R0):
        psums = {}
        for half in range(2):
            for co in range(4):
                psums[(half, co)] = cps.tile([128, 8, 64], f32,
                                             tag=f"c{half}{co}", name=f"c{half}{co}")
        for cb in range(n_cb):
            wtl = wp.tile([128, 9, 512], bf16, tag="w")
            if g == 0 and cb == 0:
                nc.sync.dma_start(wtl[:, 0:3, :], w_dram[cb][:, 0:3, :])
                nc.sync.dma_start(wtl[:, 3:9, :], w_dram[cb][:, 3:9, :])
            else:
                nc.sync.dma_start(wtl[:], w_dram[cb])
            sflat, base_slot = src_getter(g, cb)
            for tap in range(9):
                dy, dx = tap // 3 - 1, tap % 3 - 1
                for co in range(4):
                    for half in range(2):
                        off = 1 + base_slot(r0p + 8 * half + dy) * S + dx
                        rhs = sflat[:, off:off + 8 * S].rearrange(
                            "p (r c) -> p r c", c=S)[:, :, 0:64]
                        nc.tensor.matmul(
                            psums[(half, co)][:],
                            wtl[:, tap, co * 128:(co + 1) * 128],
                            rhs,
                            start=(cb == 0 and tap == 0),
                            stop=(cb == n_cb - 1 and tap == 8))
        for half in range(2):
            r0 = r0p + 8 * half
            for co in range(4):
                ps = psums[(half, co)]
                if dst_sbuf is not None:
                    dst = _rows(dst_sbuf[co])[:, 1 + r0:1 + r0 + 8, 0:64]
                else:
                    dst = dst_flat[co][:, r0:r0 + 8, :]
                nc.scalar.activation(dst, ps[:], AF.Relu, bias=bias_sb[co], scale=1.0)


def _cca(C, src_in, src_out):
    """Criss-cross attention: src_out = gamma*(outh+outw+v_b) + src_in."""
    nc, tc = C['nc'], C['tc']
    gamma, ident, identf = C['gamma'], C['ident'], C['identf']
    with contextlib.ExitStack() as es:
        atp = es.enter_context(tc.tile_pool(name="atp", bufs=1))
        ATh = atp.tile([64, 64, 64], bf16, tag="ATh")   # [j, w, h]
        ATw = atp.tile([64, 64, 64], bf16, tag="ATw")   # [j, h, w]
        eap = es.enter_context(tc.tile_pool(name="eap", bufs=1))
        EH = eap.tile([64, 64, 64], f32, tag="EH")      # [h, w, j]
        EW = eap.tile([64, 64, 64], f32, tag="EW")      # [w, h, j]
        RSH = eap.tile([64, 64], f32, tag="RSH")
        RSW = eap.tile([64, 64], f32, tag="RSW")
        Ssm = eap.tile([64, 64], f32, tag="Ssm")
        RIh = eap.tile([64, 64], f32, tag="RIh")
        RIw = eap.tile([64, 64], f32, tag="RIw")
        vtp = es.enter_context(tc.tile_pool(name="vtp", bufs=8))
        psV = es.enter_context(tc.tile_pool(name="psV", bufs=2, space="PSUM"))

        def vt_w_chunk(wc):
            VT = vtp.tile([64, 4, 512], bf16, tag="VT")
            for i in range(4):
                w = wc * 4 + i
                ps = psV.tile([64, 512], f32, tag="vps")
                for cb in range(4):
                    nc.tensor.matmul(ps[:], _rows(src_in[cb])[:, 1:65, w],
                                     C['wv'][cb][:],
                                     start=(cb == 0), stop=(cb == 3))
                (nc.scalar.activation if i % 2 else nc.vector.tensor_copy)(
                    *((VT[:, i, :], ps[:], AF.Copy) if i % 2
                      else (VT[:, i, :], ps[:])))
            return VT

        def vt_h_chunk(hc):
            VT = vtp.tile([64, 4, 512], bf16, tag="VT")
            for i in range(4):
                h = hc * 4 + i
                ps = psV.tile([64, 512], f32, tag="vps")
                for cb in range(4):
                    nc.tensor.matmul(ps[:], _rows(src_in[cb])[:, h + 1, 0:64],
                                     C['wv'][cb][:],
                                     start=(cb == 0), stop=(cb == 3))
                (nc.scalar.activation if i % 2 else nc.vector.tensor_copy)(
                    *((VT[:, i, :], ps[:], AF.Copy) if i % 2
                      else (VT[:, i, :], ps[:])))
            return VT

        # ---- phase A: q/k convs + energies
        with contextlib.ExitStack() as esA:
            qkp = esA.enter_context(tc.tile_pool(name="qkp", bufs=1))
            psQ = esA.enter_context(tc.tile_pool(name="psQ", bufs=3, space="PSUM"))
            psE = esA.enter_context(tc.tile_pool(name="psE", bufs=2, space="PSUM"))
            q_sb = qkp.tile([64, 64, 65], bf16, tag="q")
            k_sb = qkp.tile([64, 64, 65], bf16, tag="k")
            for dst_sb, wgt, bias in [(q_sb, C['wq'], C['bq']),
                                      (k_sb, C['wk'], C['bk'])]:
                dflat = dst_sb[:].rearrange("p r c -> p (r c)")
                for off, n in QK_TILES:
                    ps = psQ.tile([64, 512], f32, tag="qkps")
                    for cb in range(4):
                        rhs = src_in[cb][:, IMG0 + off:IMG0 + off + n]
                        nc.tensor.matmul(ps[:, 0:n], wgt[cb][:], rhs,
                                         start=(cb == 0), stop=(cb == 3))
                    nc.scalar.activation(dflat[:, off:off + n], ps[:, 0:n],
                                         AF.Identity, bias=bias[:], scale=1.0)
            for wi in range(16):
                ps = psE.tile([64, 4, 64], f32, tag="e4")
                for k in range(4):
                    w = wi * 4 + k
                    nc.tensor.matmul(ps[:, k, :], q_sb[:, :, w], k_sb[:, :, w],
                                     start=True, stop=True)
                nc.vector.tensor_add(EH[:, wi * 4:wi * 4 + 4, :], ps[:],
                                     C['mask4'][:])
            for hi in range(16):
                ps = psE.tile([64, 4, 64], f32, tag="e4")
                for k in range(4):
                    h = hi * 4 + k
                    nc.tensor.matmul(ps[:, k, :], q_sb[:, h, 0:64],
                                     k_sb[:, h, 0:64], start=True, stop=True)
                nc.vector.tensor_copy(EW[:, hi * 4:hi * 4 + 4, :], ps[:])

        # ---- early VT (w-orientation) chunks 0..7: keeps PE busy in softmax
        vt_cache = {wc: vt_w_chunk(wc) for wc in range(8)}

        # ---- softmax (batched) + transposes
        with contextlib.ExitStack() as esS:
            ebp = esS.enter_context(tc.tile_pool(name="ebp", bufs=2))
            psS = esS.enter_context(tc.tile_pool(name="psS", bufs=2, space="PSUM"))
            psT = esS.enter_context(tc.tile_pool(name="psT", bufs=2, space="PSUM"))
            ehf = EH[:].rearrange("p a b -> p (a b)")
            ewf = EW[:].rearrange("p a b -> p (a b)")
            nc.scalar.activation(ehf, ehf, AF.Exp)
            nc.scalar.activation(ewf, ewf, AF.Exp)
            nc.vector.reduce_sum(RSH[:], EH[:], axis=AX.X)
            nc.vector.reduce_sum(RSW[:], EW[:], axis=AX.X)
            pst = psS.tile([64, 64], f32, tag="trS")
            nc.tensor.transpose(pst[:], RSW[:], identf[:])
            nc.vector.tensor_add(Ssm[:], RSH[:], pst[:])
            nc.vector.reciprocal(RIh[:], Ssm[:])
            pst2 = psS.tile([64, 64], f32, tag="trS")
            nc.tensor.transpose(pst2[:], Ssm[:], identf[:])
            nc.vector.reciprocal(RIw[:], pst2[:])
            for wi in range(16):
                eb = ebp.tile([64, 4, 64], bf16, tag="eb")
                pt = psT.tile([64, 4, 64], bf16, tag="at")
                for k in range(4):
                    w = wi * 4 + k
                    nc.scalar.activation(eb[:, k, :], EH[:, w, :], AF.Copy,
                                         scale=RIh[:, w:w + 1])
                    nc.tensor.transpose(pt[:, k, :], eb[:, k, :], ident[:])
                nc.scalar.activation(ATh[:, wi * 4:wi * 4 + 4, :], pt[:], AF.Copy)
            for hi in range(16):
                eb = ebp.tile([64, 4, 64], bf16, tag="eb")
                pt = psT.tile([64, 4, 64], bf16, tag="at")
                for k in range(4):
                    h = hi * 4 + k
                    nc.scalar.activation(eb[:, k, :], EW[:, h, :], AF.Copy,
                                         scale=RIw[:, h:h + 1])
                    nc.tensor.transpose(pt[:, k, :], eb[:, k, :], ident[:])
                nc.scalar.activation(ATw[:, hi * 4:hi * 4 + 4, :], pt[:], AF.Copy)

        # ---- phase C
        with contextlib.ExitStack() as esC:
            sgp = esC.enter_context(tc.tile_pool(name="sgp", bufs=3))
            psD = esC.enter_context(tc.tile_pool(name="psD", bufs=6, space="PSUM"))
            # w-phase: src_out = src_in + gamma*out_h
            for wc in range(16):
                VT = vt_cache.pop(wc) if wc in vt_cache else vt_w_chunk(wc)
                for cb in range(4):
                    pso = psD.tile([128, 4, 64], f32, tag="ops")
                    for i in range(4):
                        w = wc * 4 + i
                        nc.tensor.matmul(
                            pso[:, i, :], VT[:, i, cb * 128:(cb + 1) * 128],
                            ATh[:, w, :], start=True, stop=True)
                    stg = sgp.tile([128, 4, 64], bf16, tag="stg")
                    nc.scalar.activation(stg[:], pso[:], AF.Copy, scale=gamma)
                    o_sl = _rows(src_out[cb])[:, 1:65, wc * 4:wc * 4 + 4]
                    i_sl = _rows(src_in[cb])[:, 1:65, wc * 4:wc * 4 + 4]
                    nc.vector.tensor_add(o_sl, i_sl,
                                         stg[:].rearrange("p w h -> p h w"))
            # h-phase: src_out += gamma*out_w + gamma*v_b
            for hc in range(16):
                VT = vt_h_chunk(hc)
                for cb in range(4):
                    pso = psD.tile([128, 4, 64], f32, tag="ops")
                    for i in range(4):
                        h = hc * 4 + i
                        nc.tensor.matmul(
                            pso[:, i, :], VT[:, i, cb * 128:(cb + 1) * 128],
                            ATw[:, h, :], start=True, stop=True)
                    stg = sgp.tile([128, 4, 64], bf16, tag="stg")
                    nc.scalar.activation(stg[:], pso[:], AF.Identity,
                                         scale=gamma, bias=C['gvb'][cb][:])
                    o_sl = _rows(src_out[cb])[:, 1 + hc * 4:1 + hc * 4 + 4, 0:64]
                    nc.vector.tensor_add(o_sl, o_sl, stg[:])


_BUILD_CACHE = {}


def _get_nc(gamma):
    key = round(float(gamma), 12)
    if key not in _BUILD_CACHE:
        _BUILD_CACHE[key] = build(gamma, n_reps=1)
    return _BUILD_CACHE[key]


def kernel(**inputs):
    from concourse.bass_utils import run_bass_kernel_spmd
    inputs_np = {k: np.asarray(v) for k, v in inputs.items()}
    dev, gamma = host_prep(inputs_np)
    nc = _get_nc(gamma)
    in_maps = []
    for core in range(8):
        m = dict(dev)
        m['x'] = pad_x_host(np.asarray(inputs_np['x'][core], np.float32))
        in_maps.append(m)
    res = run_bass_kernel_spmd(nc, in_maps, core_ids=list(range(8)))
    out = np.stack([r['y'].reshape(104, 64, 64) for r in res.results])
    return out.astype(np.float32)


# revision 13
# speedup vs baseline: 1.0120x; 1.0120x over previous
"""Trainium2 Bass kernel for CCHead (criss-cross attention head).

Self-contained: kernel(**inputs) takes the full unsharded inputs
(x[8, 2048, 64, 64] + weights), shards batch across 8 NeuronCores
(1 image per core, all params replicated), and returns the full
output [8, 104, 64, 64] float32.

Design: all-bf16 matmuls (PSUM fp32), x pre-padded on host so every conv
window is one contiguous DMA, src ping-pong entirely in SBUF (no DRAM
roundtrips between stages), convs as 4 passes of 16 rows with [128,8,64]
psum tiles, split-EH/EW attention energies (no partition-collapse DMAs),
batched softmax with DVE row-sum reduction, and early V-transpose chunk
emission so the PE stays busy during softmax.
"""
import contextlib
import numpy as np
import ml_dtypes
import concourse.bass as bass
import concourse.tile as tile
from concourse import bacc, mybir

f32 = mybir.dt.float32
bf16 = mybir.dt.bfloat16
AF = mybir.ActivationFunctionType
AX = mybir.AxisListType
BF_NP = ml_dtypes.bfloat16

S = 65
NR = 67
FLAT = NR * S + 2          # 4357 (src tiles: lead pad + 67 padded rows + slack)
IMG0 = 1 + S               # flat offset of image row 0, col 0
XFLAT = 74 * S + 2         # padded x in DRAM
WLEN = 18 * S + 2          # conv window: 18 padded row slots + slack
X_DEV_SHAPE = (16, 128, XFLAT)
PASS_R0 = [0, 16, 32, 48]
QK_TILES = [(i * 512, 512) for i in range(8)] + [(4096, 64)]


def pad_x_host(x_core):
    """[2048, 64, 64] f32 -> [16, 128, XFLAT] bf16 padded flat."""
    xb = x_core.reshape(16, 128, 64, 64).astype(BF_NP)
    arr = np.zeros((16, 128, XFLAT), BF_NP)
    arr[:, :, 1:1 + NR * S].reshape(16, 128, NR, S)[:, :, 1:65, 0:64] = xb
    return arr


def host_prep(inputs):
    f = np.float32

    def fold(w, g, b, m, v):
        s = (g / np.sqrt(v + 1e-5)).astype(f)
        return (w * s[:, None, None, None]).astype(f), (b - m * s).astype(f)

    def wt_dev(w):  # [co, ci, 3, 3] -> [nci, 128, 9, co] bf16
        co, ci = w.shape[:2]
        return np.ascontiguousarray(
            w.reshape(co, ci, 9).transpose(1, 2, 0).reshape(
                ci // 128, 128, 9, co).astype(BF_NP))

    def t1x1(w):  # [co, ci, 1, 1] -> [nci, 128, co] bf16
        co, ci = w.shape[:2]
        return np.ascontiguousarray(
            w.reshape(co, ci).T.reshape(ci // 128, 128, co).astype(BF_NP))

    wa, ba = fold(inputs['conva_w'], inputs['conva_g'], inputs['conva_b'],
                  inputs['conva_m'], inputs['conva_v'])
    wb, bb = fold(inputs['convb_w'], inputs['convb_g'], inputs['convb_b'],
                  inputs['convb_m'], inputs['convb_v'])
    wt, bt = fold(inputs['bott_w'], inputs['bott_g'], inputs['bott_b'],
                  inputs['bott_m'], inputs['bott_v'])
    gamma = float(np.asarray(inputs['cc_gamma']).reshape(-1)[0])
    mask = np.zeros((64, 64), f)
    np.fill_diagonal(mask, -1e30)
    mask4 = np.ascontiguousarray(
        np.broadcast_to(mask[:, None, :], (64, 4, 64)).astype(f))
    dev = {
        'wa': wt_dev(wa), 'ba': ba.reshape(4, 128, 1),
        'wb': wt_dev(wb), 'bb': bb.reshape(4, 128, 1),
        'wt': wt_dev(wt), 'bt': bt.reshape(4, 128, 1),
        'wc': t1x1(inputs['cls_w']),
        'bc': inputs['cls_b'].astype(f).reshape(104, 1),
        'wq': t1x1(inputs['q_w']), 'bq': inputs['q_b'].astype(f).reshape(64, 1),
        'wk': t1x1(inputs['k_w']), 'bk': inputs['k_b'].astype(f).reshape(64, 1),
        'wv': t1x1(inputs['v_w']),
        'gvb': (gamma * inputs['v_b']).astype(f).reshape(4, 128, 1),
        'mask4': mask4,
        'ident': np.eye(64, dtype=BF_NP),
        'identf': np.eye(64, dtype=f),
    }
    return dev, gamma


INPUT_SPECS = [
    ('wa', [16, 128, 9, 512], bf16), ('ba', [4, 128, 1], f32),
    ('wb', [4, 128, 9, 512], bf16), ('bb', [4, 128, 1], f32),
    ('wt', [20, 128, 9, 512], bf16), ('bt', [4, 128, 1], f32),
    ('wc', [4, 128, 104], bf16), ('bc', [104, 1], f32),
    ('wq', [4, 128, 64], bf16), ('bq', [64, 1], f32),
    ('wk', [4, 128, 64], bf16), ('bk', [64, 1], f32),
    ('wv', [4, 128, 512], bf16),
    ('gvb', [4, 128, 1], f32),
    ('mask4', [64, 4, 64], f32),
    ('ident', [64, 64], bf16),
    ('identf', [64, 64], f32),
]


def build(gamma, n_reps=1):
    nc = bacc.Bacc("TRN2", num_devices=8)
    t = {'x': nc.dram_tensor("x", list(X_DEV_SHAPE), bf16, kind="ExternalInput")}
    for nm, shape, dt in INPUT_SPECS:
        t[nm] = nc.dram_tensor(nm, shape, dt, kind="ExternalInput")
    y = nc.dram_tensor("y", [104, 64, 64], f32, kind="ExternalOutput")
    with tile.TileContext(nc) as tc:
        _build_body(tc, t, y, gamma, n_reps)
    nc.compile()
    return nc


def _rows(flat_tile):
    """[128, FLAT] -> padded row view [128, 67, 65] (skips lead pad elem)."""
    return flat_tile[:, 1:1 + NR * S].rearrange("p (r c) -> p r c", c=S)


def _build_body(tc, t, y, gamma, n_reps):
    nc = tc.nc
    with contextlib.ExitStack() as est:
        cp = est.enter_context(tc.tile_pool(name="const", bufs=1))

        def cload(nm, shape, dt):
            tl = cp.tile(shape, dt, tag=nm, name=nm)
            nc.sync.dma_start(tl[:], t[nm][:])
            return tl

        def load_blocks(nm, n, shape, dt=f32):
            out = []
            for i in range(n):
                tl = cp.tile(shape, dt, tag=f"{nm}{i}", name=f"{nm}{i}")
                nc.sync.dma_start(tl[:], t[nm][i])
                out.append(tl)
            return out

        C = dict(nc=nc, tc=tc, t=t, y=y, gamma=gamma,
                 ident=cload('ident', [64, 64], bf16),
                 identf=cload('identf', [64, 64], f32),
                 mask4=cload('mask4', [64, 4, 64], f32),
                 bq=cload('bq', [64, 1], f32),
                 bk=cload('bk', [64, 1], f32),
                 bc=cload('bc', [104, 1], f32),
                 bias_a=load_blocks('ba', 4, [128, 1]),
                 bias_b=load_blocks('bb', 4, [128, 1]),
                 bias_t=load_blocks('bt', 4, [128, 1]),
                 gvb=load_blocks('gvb', 4, [128, 1]),
                 wq=load_blocks('wq', 4, [128, 64], bf16),
                 wk=load_blocks('wk', 4, [128, 64], bf16),
                 wv=load_blocks('wv', 4, [128, 512], bf16),
                 wc=load_blocks('wc', 4, [128, 104], bf16))

        ap = est.enter_context(tc.tile_pool(name="actp", bufs=1))
        srcA = [ap.tile([128, FLAT], bf16, tag=f"sa{i}", name=f"sa{i}")
                for i in range(4)]
        srcB = [ap.tile([128, FLAT], bf16, tag=f"sb{i}", name=f"sb{i}")
                for i in range(4)]
        for blk in srcA + srcB:
            nc.any.memset(blk[:], 0.0)
        C['srcA'], C['srcB'] = srcA, srcB

        for _ in range(n_reps):
            _network(C)


def _network(C):
    nc, tc, t = C['nc'], C['tc'], C['t']
    srcA, srcB = C['srcA'], C['srcB']
    # conva: x windows -> srcA
    with contextlib.ExitStack() as es:
        wp = es.enter_context(tc.tile_pool(name="wp", bufs=4))
        cps = es.enter_context(tc.tile_pool(name="cps", bufs=1, space="PSUM"))
        xsp = es.enter_context(tc.tile_pool(name="xsp", bufs=3))
        xg = _x_win_getter(C, xsp)
        _conv3x3(C, wp, cps, xg, 16, t['wa'], C['bias_a'], dst_sbuf=srcA)
    # CCA 1: srcA -> srcB;  CCA 2: srcB -> srcA
    _cca(C, srcA, srcB)
    _cca(C, srcB, srcA)
    # convb: srcA -> srcB
    with contextlib.ExitStack() as es:
        wp = es.enter_context(tc.tile_pool(name="wpb", bufs=3))
        cps = es.enter_context(tc.tile_pool(name="cpsb", bufs=1, space="PSUM"))
        _conv3x3(C, wp, cps, _src_getter(srcA), 4, t['wb'], C['bias_b'],
                 dst_sbuf=srcB)
    # bott: x windows (16cb) + srcB (4cb) -> ot (SBUF flat)
    with contextlib.ExitStack() as eso:
        otp = eso.enter_context(tc.tile_pool(name="otp", bufs=1))
        ot = [otp.tile([128, 64, 64], bf16, tag=f"ot{i}", name=f"ot{i}")
              for i in range(4)]
        with contextlib.ExitStack() as es:
            wp = es.enter_context(tc.tile_pool(name="wpt", bufs=4))
            cps = es.enter_context(tc.tile_pool(name="cpst", bufs=1, space="PSUM"))
            xsp = es.enter_context(tc.tile_pool(name="xspt", bufs=3))
            xg = _x_win_getter(C, xsp)
            sg = _src_getter(srcB)

            def src_get(g, cb):
                return xg(g, cb) if cb < 16 else sg(g, cb - 16)

            _conv3x3(C, wp, cps, src_get, 20, t['wt'], C['bias_t'], dst_flat=ot)
        # cls: ot (SBUF) -> y
        es = eso
        cop = es.enter_context(tc.tile_pool(name="cop", bufs=1))
        cpp = es.enter_context(tc.tile_pool(name="cpp", bufs=2, space="PSUM"))
        out_sb = cop.tile([104, 64, 64], f32)
        oflat = out_sb[:].rearrange("p r c -> p (r c)")
        for off, n in [(i * 512, 512) for i in range(8)]:
            ps = cpp.tile([104, 512], f32, tag="clsps")
            for cb in range(4):
                rhs = ot[cb][:].rearrange("p r c -> p (r c)")[:, off:off + n]
                nc.tensor.matmul(ps[:, 0:n], C['wc'][cb][:], rhs,
                                 start=(cb == 0), stop=(cb == 3))
            nc.scalar.activation(oflat[:, off:off + n], ps[:, 0:n], AF.Identity,
                                 bias=C['bc'][:], scale=1.0)
        nc.sync.dma_start(C['y'][:], out_sb[:])


def _x_win_getter(C, xsp):
    nc, t = C['nc'], C['t']
    cache = {}

    def get(g, cb):
        key = (g, cb)
        if key in cache:
            return cache[key]
        r0p = PASS_R0[g]
        xs = xsp.tile([128, WLEN], bf16, tag="xs")
        nc.sync.dma_start(xs[:], t['x'][cb][:, r0p * S:r0p * S + WLEN])
        res = (xs, lambda rr, _p=r0p: rr + 1 - _p)
        cache[key] = res
        return res

    return get


def _src_getter(src):
    def get(g, cb):
        return (src[cb], lambda rr: rr + 1)
    return get


def _conv3x3(C, wp, cps, src_getter, n_cb, w_dram, bias_sb,
             dst_sbuf=None, dst_flat=None):
    nc = C['nc']
    for g, r0p in enumerate(PASS_R0):
        psums = {}
        for half in range(2):
            for co in range(4):
                psums[(half, co)] = cps.tile([128, 8, 64], f32,
                                             tag=f"c{half}{co}", name=f"c{half}{co}")
        for cb in range(n_cb):
            wtl = wp.tile([128, 9, 512], bf16, tag="w")
            nc.sync.dma_start(wtl[:], w_dram[cb])
            sflat, base_slot = src_getter(g, cb)
            for tap in range(9):
                dy, dx = tap // 3 - 1, tap % 3 - 1
                for co in range(4):
                    for half in range(2):
                        off = 1 + base_slot(r0p + 8 * half + dy) * S + dx
                        rhs = sflat[:, off:off + 8 * S].rearrange(
                            "p (r c) -> p r c", c=S)[:, :, 0:64]
                        nc.tensor.matmul(
                            psums[(half, co)][:],
                            wtl[:, tap, co * 128:(co + 1) * 128],
                            rhs,
                            start=(cb == 0 and tap == 0),
                            stop=(cb == n_cb - 1 and tap == 8))
        for half in range(2):
            r0 = r0p + 8 * half
            for co in range(4):
                ps = psums[(half, co)]
                if dst_sbuf is not None:
                    dst = _rows(dst_sbuf[co])[:, 1 + r0:1 + r0 + 8, 0:64]
                else:
                    dst = dst_flat[co][:, r0:r0 + 8, :]
                nc.scalar.activation(dst, ps[:], AF.Relu, bias=bias_sb[co], scale=1.0)


def _cca(C, src_in, src_out):
    """Criss-cross attention: src_out = gamma*(outh+outw+v_b) + src_in."""
    nc, tc = C['nc'], C['tc']
    gamma, ident, identf = C['gamma'], C['ident'], C['identf']
    with contextlib.ExitStack() as es:
        atp = es.enter_context(tc.tile_pool(name="atp", bufs=1))
        ATh = atp.tile([64, 64, 64], bf16, tag="ATh")   # [j, w, h]
        ATw = atp.tile([64, 64, 64], bf16, tag="ATw")   # [j, h, w]
        eap = es.enter_context(tc.tile_pool(name="eap", bufs=1))
        EH = eap.tile([64, 64, 64], f32, tag="EH")      # [h, w, j]
        EW = eap.tile([64, 64, 64], f32, tag="EW")      # [w, h, j]
        RSH = eap.tile([64, 64], f32, tag="RSH")
        RSW = eap.tile([64, 64], f32, tag="RSW")
        Ssm = eap.tile([64, 64], f32, tag="Ssm")
        RIh = eap.tile([64, 64], f32, tag="RIh")
        RIw = eap.tile([64, 64], f32, tag="RIw")
        vtp = es.enter_context(tc.tile_pool(name="vtp", bufs=8))
        psV = es.enter_context(tc.tile_pool(name="psV", bufs=2, space="PSUM"))

        def vt_w_chunk(wc):
            VT = vtp.tile([64, 4, 512], bf16, tag="VT")
            for i in range(4):
                w = wc * 4 + i
                ps = psV.tile([64, 512], f32, tag="vps")
                for cb in range(4):
                    nc.tensor.matmul(ps[:], _rows(src_in[cb])[:, 1:65, w],
                                     C['wv'][cb][:],
                                     start=(cb == 0), stop=(cb == 3))
                (nc.scalar.activation if i % 2 else nc.vector.tensor_copy)(
                    *((VT[:, i, :], ps[:], AF.Copy) if i % 2
                      else (VT[:, i, :], ps[:])))
            return VT

        def vt_h_chunk(hc):
            VT = vtp.tile([64, 4, 512], bf16, tag="VT")
            for i in range(4):
                h = hc * 4 + i
                ps = psV.tile([64, 512], f32, tag="vps")
                for cb in range(4):
                    nc.tensor.matmul(ps[:], _rows(src_in[cb])[:, h + 1, 0:64],
                                     C['wv'][cb][:],
                                     start=(cb == 0), stop=(cb == 3))
                (nc.scalar.activation if i % 2 else nc.vector.tensor_copy)(
                    *((VT[:, i, :], ps[:], AF.Copy) if i % 2
                      else (VT[:, i, :], ps[:])))
            return VT

        # ---- phase A: q/k convs + energies
        with contextlib.ExitStack() as esA:
            qkp = esA.enter_context(tc.tile_pool(name="qkp", bufs=1))
            psQ = esA.enter_context(tc.tile_pool(name="psQ", bufs=2, space="PSUM"))
            psE = esA.enter_context(tc.tile_pool(name="psE", bufs=2, space="PSUM"))
            q_sb = qkp.tile([64, 64, 65], bf16, tag="q")
            k_sb = qkp.tile([64, 64, 65], bf16, tag="k")
            for dst_sb, wgt, bias in [(q_sb, C['wq'], C['bq']),
                                      (k_sb, C['wk'], C['bk'])]:
                dflat = dst_sb[:].rearrange("p r c -> p (r c)")
                for off, n in QK_TILES:
                    ps = psQ.tile([64, 512], f32, tag="qkps")
                    for cb in range(4):
                        rhs = src_in[cb][:, IMG0 + off:IMG0 + off + n]
                        nc.tensor.matmul(ps[:, 0:n], wgt[cb][:], rhs,
                                         start=(cb == 0), stop=(cb == 3))
                    nc.scalar.activation(dflat[:, off:off + n], ps[:, 0:n],
                                         AF.Identity, bias=bias[:], scale=1.0)
            for wi in range(16):
                ps = psE.tile([64, 4, 64], f32, tag="e4")
                for k in range(4):
                    w = wi * 4 + k
                    nc.tensor.matmul(ps[:, k, :], q_sb[:, :, w], k_sb[:, :, w],
                                     start=True, stop=True)
                nc.vector.tensor_add(EH[:, wi * 4:wi * 4 + 4, :], ps[:],
                                     C['mask4'][:])
            for hi in range(16):
                ps = psE.tile([64, 4, 64], f32, tag="e4")
                for k in range(4):
                    h = hi * 4 + k
                    nc.tensor.matmul(ps[:, k, :], q_sb[:, h, 0:64],
                                     k_sb[:, h, 0:64], start=True, stop=True)
                nc.vector.tensor_copy(EW[:, hi * 4:hi * 4 + 4, :], ps[:])

        # ---- early VT (w-orientation) chunks 0..7: keeps PE busy in softmax
        vt_cache = {wc: vt_w_chunk(wc) for wc in range(8)}

        # ---- softmax (batched) + transposes
        with contextlib.ExitStack() as esS:
            ebp = esS.enter_context(tc.tile_pool(name="ebp", bufs=2))
            psS = esS.enter_context(tc.tile_pool(name="psS", bufs=2, space="PSUM"))
            psT = esS.enter_context(tc.tile_pool(name="psT", bufs=2, space="PSUM"))
            ehf = EH[:].rearrange("p a b -> p (a b)")
            ewf = EW[:].rearrange("p a b -> p (a b)")
            nc.scalar.activation(ehf, ehf, AF.Exp)
            nc.scalar.activation(ewf, ewf, AF.Exp)
            nc.vector.reduce_sum(RSH[:], EH[:], axis=AX.X)
            nc.vector.reduce_sum(RSW[:], EW[:], axis=AX.X)
            pst = psS.tile([64, 64], f32, tag="trS")
            nc.tensor.transpose(pst[:], RSW[:], identf[:])
            nc.vector.tensor_add(Ssm[:], RSH[:], pst[:])
            nc.vector.reciprocal(RIh[:], Ssm[:])
            pst2 = psS.tile([64, 64], f32, tag="trS")
            nc.tensor.transpose(pst2[:], Ssm[:], identf[:])
            nc.vector.reciprocal(RIw[:], pst2[:])
            for wi in range(16):
                eb = ebp.tile([64, 4, 64], bf16, tag="eb")
                pt = psT.tile([64, 4, 64], bf16, tag="at")
                for k in range(4):
                    w = wi * 4 + k
                    nc.scalar.activation(eb[:, k, :], EH[:, w, :], AF.Copy,
                                         scale=RIh[:, w:w + 1])
                    nc.tensor.transpose(pt[:, k, :], eb[:, k, :], ident[:])
                nc.scalar.activation(ATh[:, wi * 4:wi * 4 + 4, :], pt[:], AF.Copy)
            for hi in range(16):
                eb = ebp.tile([64, 4, 64], bf16, tag="eb")
                pt = psT.tile([64, 4, 64], bf16, tag="at")
                for k in range(4):
                    h = hi * 4 + k
                    nc.scalar.activation(eb[:, k, :], EW[:, h, :], AF.Copy,
                                         scale=RIw[:, h:h + 1])
                    nc.tensor.transpose(pt[:, k, :], eb[:, k, :], ident[:])
                nc.scalar.activation(ATw[:, hi * 4:hi * 4 + 4, :], pt[:], AF.Copy)

        # ---- phase C
        with contextlib.ExitStack() as esC:
            sgp = esC.enter_context(tc.tile_pool(name="sgp", bufs=3))
            psD = esC.enter_context(tc.tile_pool(name="psD", bufs=6, space="PSUM"))
            # w-phase: src_out = src_in + gamma*out_h
            for wc in range(16):
                VT = vt_cache.pop(wc) if wc in vt_cache else vt_w_chunk(wc)
                for cb in range(4):
                    pso = psD.tile([128, 4, 64], f32, tag="ops")
                    for i in range(4):
                        w = wc * 4 + i
                        nc.tensor.matmul(
                            pso[:, i, :], VT[:, i, cb * 128:(cb + 1) * 128],
                            ATh[:, w, :], start=True, stop=True)
                    stg = sgp.tile([128, 4, 64], bf16, tag="stg")
                    nc.scalar.activation(stg[:], pso[:], AF.Copy, scale=gamma)
                    o_sl = _rows(src_out[cb])[:, 1:65, wc * 4:wc * 4 + 4]
                    i_sl = _rows(src_in[cb])[:, 1:65, wc * 4:wc * 4 + 4]
                    nc.vector.tensor_add(o_sl, i_sl,
                                         stg[:].rearrange("p w h -> p h w"))
            # h-phase: src_out += gamma*out_w + gamma*v_b
            for hc in range(16):
                VT = vt_h_chunk(hc)
                for cb in range(4):
                    pso = psD.tile([128, 4, 64], f32, tag="ops")
                    for i in range(4):
                        h = hc * 4 + i
                        nc.tensor.matmul(
                            pso[:, i, :], VT[:, i, cb * 128:(cb + 1) * 128],
                            ATw[:, h, :], start=True, stop=True)
                    stg = sgp.tile([128, 4, 64], bf16, tag="stg")
                    nc.scalar.activation(stg[:], pso[:], AF.Identity,
                                         scale=gamma, bias=C['gvb'][cb][:])
                    o_sl = _rows(src_out[cb])[:, 1 + hc * 4:1 + hc * 4 + 4, 0:64]
                    nc.vector.tensor_add(o_sl, o_sl, stg[:])


_BUILD_CACHE = {}


def _get_nc(gamma):
    key = round(float(gamma), 12)
    if key not in _BUILD_CACHE:
        _BUILD_CACHE[key] = build(gamma, n_reps=1)
    return _BUILD_CACHE[key]


def kernel(**inputs):
    from concourse.bass_utils import run_bass_kernel_spmd
    inputs_np = {k: np.asarray(v) for k, v in inputs.items()}
    dev, gamma = host_prep(inputs_np)
    nc = _get_nc(gamma)
    in_maps = []
    for core in range(8):
        m = dict(dev)
        m['x'] = pad_x_host(np.asarray(inputs_np['x'][core], np.float32))
        in_maps.append(m)
    res = run_bass_kernel_spmd(nc, in_maps, core_ids=list(range(8)))
    out = np.stack([r['y'].reshape(104, 64, 64) for r in res.results])
    return out.astype(np.float32)


# revision 14
# speedup vs baseline: 1.1107x; 1.0976x over previous
"""Trainium2 Bass kernel for CCHead (criss-cross attention head).

Self-contained: kernel(**inputs) takes the full unsharded inputs
(x[8, 2048, 64, 64] + weights), shards batch across 8 NeuronCores
(1 image per core, all params replicated), and returns the full
output [8, 104, 64, 64] float32.

Design: all-bf16 matmuls (PSUM fp32), x pre-padded on host so every conv
window is one contiguous DMA, src ping-pong entirely in SBUF (no DRAM
roundtrips between stages), convs as 4 passes of 16 rows with [128,8,64]
psum tiles, split-EH/EW attention energies (no partition-collapse DMAs),
batched softmax with DVE row-sum reduction, and early V-transpose chunk
emission so the PE stays busy during softmax.
"""
import contextlib
import numpy as np
import ml_dtypes
import concourse.bass as bass
import concourse.tile as tile
from concourse import bacc, mybir

f32 = mybir.dt.float32
bf16 = mybir.dt.bfloat16
AF = mybir.ActivationFunctionType
AX = mybir.AxisListType
BF_NP = ml_dtypes.bfloat16

S = 65
NR = 67
FLAT = NR * S + 2          # 4357 (src tiles: lead pad + 67 padded rows + slack)
IMG0 = 1 + S               # flat offset of image row 0, col 0
XFLAT = 74 * S + 2         # padded x in DRAM
WLEN = 18 * S + 2          # conv window: 18 padded row slots + slack
X_DEV_SHAPE = (16, 128, XFLAT)
PASS_R0 = [0, 16, 32, 48]
QK_TILES = [(i * 512, 512) for i in range(8)] + [(4096, 64)]


def pad_x_host(x_core):
    """[2048, 64, 64] f32 -> [16, 128, XFLAT] bf16 padded flat."""
    xb = x_core.reshape(16, 128, 64, 64).astype(BF_NP)
    arr = np.zeros((16, 128, XFLAT), BF_NP)
    arr[:, :, 1:1 + NR * S].reshape(16, 128, NR, S)[:, :, 1:65, 0:64] = xb
    return arr


def host_prep(inputs):
    f = np.float32

    def fold(w, g, b, m, v):
        s = (g / np.sqrt(v + 1e-5)).astype(f)
        return (w * s[:, None, None, None]).astype(f), (b - m * s).astype(f)

    def wt_dev(w):  # [co, ci, 3, 3] -> [nci, 128, 9, co] bf16
        co, ci = w.shape[:2]
        return np.ascontiguousarray(
            w.reshape(co, ci, 9).transpose(1, 2, 0).reshape(
                ci // 128, 128, 9, co).astype(BF_NP))

    def t1x1(w):  # [co, ci, 1, 1] -> [nci, 128, co] bf16
        co, ci = w.shape[:2]
        return np.ascontiguousarray(
            w.reshape(co, ci).T.reshape(ci // 128, 128, co).astype(BF_NP))

    wa, ba = fold(inputs['conva_w'], inputs['conva_g'], inputs['conva_b'],
                  inputs['conva_m'], inputs['conva_v'])
    wb, bb = fold(inputs['convb_w'], inputs['convb_g'], inputs['convb_b'],
                  inputs['convb_m'], inputs['convb_v'])
    wt, bt = fold(inputs['bott_w'], inputs['bott_g'], inputs['bott_b'],
                  inputs['bott_m'], inputs['bott_v'])
    gamma = float(np.asarray(inputs['cc_gamma']).reshape(-1)[0])
    mask = np.zeros((64, 64), f)
    np.fill_diagonal(mask, -1e30)
    mask4 = np.ascontiguousarray(
        np.broadcast_to(mask[:, None, :], (64, 4, 64)).astype(f))
    dev = {
        'wa': wt_dev(wa), 'ba': ba.reshape(4, 128, 1),
        'wb': wt_dev(wb), 'bb': bb.reshape(4, 128, 1),
        'wt': wt_dev(wt), 'bt': bt.reshape(4, 128, 1),
        'wc': t1x1(inputs['cls_w']),
        'bc': inputs['cls_b'].astype(f).reshape(104, 1),
        'wq': t1x1(inputs['q_w']), 'bq': inputs['q_b'].astype(f).reshape(64, 1),
        'wk': t1x1(inputs['k_w']), 'bk': inputs['k_b'].astype(f).reshape(64, 1),
        'wv': t1x1(inputs['v_w']),
        'gvb': (gamma * inputs['v_b']).astype(f).reshape(4, 128, 1),
        'mask4': mask4,
        'ident': np.eye(64, dtype=BF_NP),
        'identf': np.eye(64, dtype=f),
    }
    return dev, gamma


INPUT_SPECS = [
    ('wa', [16, 128, 9, 512], bf16), ('ba', [4, 128, 1], f32),
    ('wb', [4, 128, 9, 512], bf16), ('bb', [4, 128, 1], f32),
    ('wt', [20, 128, 9, 512], bf16), ('bt', [4, 128, 1], f32),
    ('wc', [4, 128, 104], bf16), ('bc', [104, 1], f32),
    ('wq', [4, 128, 64], bf16), ('bq', [64, 1], f32),
    ('wk', [4, 128, 64], bf16), ('bk', [64, 1], f32),
    ('wv', [4, 128, 512], bf16),
    ('gvb', [4, 128, 1], f32),
    ('mask4', [64, 4, 64], f32),
    ('ident', [64, 64], bf16),
    ('identf', [64, 64], f32),
]


def build(gamma, n_reps=1):
    nc = bacc.Bacc("TRN2", num_devices=8)
    t = {'x': nc.dram_tensor("x", list(X_DEV_SHAPE), bf16, kind="ExternalInput")}
    for nm, shape, dt in INPUT_SPECS:
        t[nm] = nc.dram_tensor(nm, shape, dt, kind="ExternalInput")
    y = nc.dram_tensor("y", [104, 64, 64], f32, kind="ExternalOutput")
    with tile.TileContext(nc) as tc:
        _build_body(tc, t, y, gamma, n_reps)
    nc.compile()
    return nc


def _rows(flat_tile):
    """[128, FLAT] -> padded row view [128, 67, 65] (skips lead pad elem)."""
    return flat_tile[:, 1:1 + NR * S].rearrange("p (r c) -> p r c", c=S)


def _build_body(tc, t, y, gamma, n_reps):
    nc = tc.nc
    with contextlib.ExitStack() as est:
        cp = est.enter_context(tc.tile_pool(name="const", bufs=1))

        def cload(nm, shape, dt):
            tl = cp.tile(shape, dt, tag=nm, name=nm)
            nc.sync.dma_start(tl[:], t[nm][:])
            return tl

        def load_blocks(nm, n, shape, dt=f32):
            out = []
            for i in range(n):
                tl = cp.tile(shape, dt, tag=f"{nm}{i}", name=f"{nm}{i}")
                nc.sync.dma_start(tl[:], t[nm][i])
                out.append(tl)
            return out

        C = dict(nc=nc, tc=tc, t=t, y=y, gamma=gamma,
                 bias_a=load_blocks('ba', 4, [128, 1]))

        def load_late():
            C.update(ident=cload('ident', [64, 64], bf16),
                     identf=cload('identf', [64, 64], f32),
                     mask4=cload('mask4', [64, 4, 64], f32),
                     bq=cload('bq', [64, 1], f32),
                     bk=cload('bk', [64, 1], f32),
                     bc=cload('bc', [104, 1], f32),
                     bias_b=load_blocks('bb', 4, [128, 1]),
                     bias_t=load_blocks('bt', 4, [128, 1]),
                     gvb=load_blocks('gvb', 4, [128, 1]),
                     wq=load_blocks('wq', 4, [128, 64], bf16),
                     wk=load_blocks('wk', 4, [128, 64], bf16),
                     wv=load_blocks('wv', 4, [128, 512], bf16),
                     wc=load_blocks('wc', 4, [128, 104], bf16))
        C['load_late'] = load_late

        ap = est.enter_context(tc.tile_pool(name="actp", bufs=1))
        srcA = [ap.tile([128, FLAT], bf16, tag=f"sa{i}", name=f"sa{i}")
                for i in range(4)]
        srcB = [ap.tile([128, FLAT], bf16, tag=f"sb{i}", name=f"sb{i}")
                for i in range(4)]
        for blk in srcA + srcB:
            nc.any.memset(blk[:], 0.0)
        C['srcA'], C['srcB'] = srcA, srcB

        for _ in range(n_reps):
            _network(C)


def _network(C):
    nc, tc, t = C['nc'], C['tc'], C['t']
    srcA, srcB = C['srcA'], C['srcB']
    # conva: x windows -> srcA
    with contextlib.ExitStack() as es:
        wp = es.enter_context(tc.tile_pool(name="wp", bufs=4))
        cps = es.enter_context(tc.tile_pool(name="cps", bufs=1, space="PSUM"))
        xsp = es.enter_context(tc.tile_pool(name="xsp", bufs=3))
        xg = _x_win_getter(C, xsp)
        _conv3x3(C, wp, cps, xg, 16, t['wa'], C['bias_a'], dst_sbuf=srcA)
    if 'load_late' in C:
        C.pop('load_late')()
    # CCA 1: srcA -> srcB;  CCA 2: srcB -> srcA
    _cca(C, srcA, srcB)
    _cca(C, srcB, srcA)
    # convb: srcA -> srcB
    with contextlib.ExitStack() as es:
        wp = es.enter_context(tc.tile_pool(name="wpb", bufs=3))
        cps = es.enter_context(tc.tile_pool(name="cpsb", bufs=1, space="PSUM"))
        _conv3x3(C, wp, cps, _src_getter(srcA), 4, t['wb'], C['bias_b'],
                 dst_sbuf=srcB)
    # bott: x windows (16cb) + srcB (4cb) -> ot (SBUF flat)
    with contextlib.ExitStack() as eso:
        otp = eso.enter_context(tc.tile_pool(name="otp", bufs=1))
        ot = [otp.tile([128, 64, 64], bf16, tag=f"ot{i}", name=f"ot{i}")
              for i in range(4)]
        with contextlib.ExitStack() as es:
            wp = es.enter_context(tc.tile_pool(name="wpt", bufs=4))
            cps = es.enter_context(tc.tile_pool(name="cpst", bufs=1, space="PSUM"))
            xsp = es.enter_context(tc.tile_pool(name="xspt", bufs=3))
            xg = _x_win_getter(C, xsp)
            sg = _src_getter(srcB)

            def src_get(g, cb):
                return xg(g, cb) if cb < 16 else sg(g, cb - 16)

            _conv3x3(C, wp, cps, src_get, 20, t['wt'], C['bias_t'], dst_flat=ot)
        # cls: ot (SBUF) -> y
        es = eso
        cop = es.enter_context(tc.tile_pool(name="cop", bufs=1))
        cpp = es.enter_context(tc.tile_pool(name="cpp", bufs=2, space="PSUM"))
        out_sb = cop.tile([104, 64, 64], f32)
        oflat = out_sb[:].rearrange("p r c -> p (r c)")
        for off, n in [(i * 512, 512) for i in range(8)]:
            ps = cpp.tile([104, 512], f32, tag="clsps")
            for cb in range(4):
                rhs = ot[cb][:].rearrange("p r c -> p (r c)")[:, off:off + n]
                nc.tensor.matmul(ps[:, 0:n], C['wc'][cb][:], rhs,
                                 start=(cb == 0), stop=(cb == 3))
            nc.scalar.activation(oflat[:, off:off + n], ps[:, 0:n], AF.Identity,
                                 bias=C['bc'][:], scale=1.0)
        nc.sync.dma_start(C['y'][:], out_sb[:])


def _x_win_getter(C, xsp):
    nc, t = C['nc'], C['t']
    cache = {}

    def get(g, cb):
        key = (g, cb)
        if key in cache:
            return cache[key]
        r0p = PASS_R0[g]
        xs = xsp.tile([128, WLEN], bf16, tag="xs")
        nc.sync.dma_start(xs[:], t['x'][cb][:, r0p * S:r0p * S + WLEN])
        res = (xs, lambda rr, _p=r0p: rr + 1 - _p)
        cache[key] = res
        return res

    return get


def _src_getter(src):
    def get(g, cb):
        return (src[cb], lambda rr: rr + 1)
    return get


def _conv3x3(C, wp, cps, src_getter, n_cb, w_dram, bias_sb,
             dst_sbuf=None, dst_flat=None):
    nc = C['nc']
    for g, r0p in enumerate(PASS_R0):
        psums = {}
        for half in range(2):
            for co in range(4):
                psums[(half, co)] = cps.tile([128, 8, 64], f32,
                                             tag=f"c{half}{co}", name=f"c{half}{co}")
        for cb in range(n_cb):
            wtl = wp.tile([128, 9, 512], bf16, tag="w")
            if g == 0 and cb == 0:
                nc.sync.dma_start(wtl[:, 0:3, :], w_dram[cb][:, 0:3, :])
                nc.sync.dma_start(wtl[:, 3:9, :], w_dram[cb][:, 3:9, :])
            else:
                nc.sync.dma_start(wtl[:], w_dram[cb])
            sflat, base_slot = src_getter(g, cb)
            for tap in range(9):
                dy, dx = tap // 3 - 1, tap % 3 - 1
                for co in range(4):
                    for half in range(2):
                        off = 1 + base_slot(r0p + 8 * half + dy) * S + dx
                        rhs = sflat[:, off:off + 8 * S].rearrange(
                            "p (r c) -> p r c", c=S)[:, :, 0:64]
                        nc.tensor.matmul(
                            psums[(half, co)][:],
                            wtl[:, tap, co * 128:(co + 1) * 128],
                            rhs,
                            start=(cb == 0 and tap == 0),
                            stop=(cb == n_cb - 1 and tap == 8))
        for half in range(2):
            r0 = r0p + 8 * half
            for co in range(4):
                ps = psums[(half, co)]
                if dst_sbuf is not None:
                    dst = _rows(dst_sbuf[co])[:, 1 + r0:1 + r0 + 8, 0:64]
                else:
                    dst = dst_flat[co][:, r0:r0 + 8, :]
                nc.scalar.activation(dst, ps[:], AF.Relu, bias=bias_sb[co], scale=1.0)


def _cca(C, src_in, src_out):
    """Criss-cross attention: src_out = gamma*(outh+outw+v_b) + src_in."""
    nc, tc = C['nc'], C['tc']
    gamma, ident, identf = C['gamma'], C['ident'], C['identf']
    with contextlib.ExitStack() as es:
        atp = es.enter_context(tc.tile_pool(name="atp", bufs=1))
        ATh = atp.tile([64, 64, 64], bf16, tag="ATh")   # [j, w, h]
        ATw = atp.tile([64, 64, 64], bf16, tag="ATw")   # [j, h, w]
        eap = es.enter_context(tc.tile_pool(name="eap", bufs=1))
        EH = eap.tile([64, 64, 64], f32, tag="EH")      # [h, w, j]
        EW = eap.tile([64, 64, 64], f32, tag="EW")      # [w, h, j]
        RSH = eap.tile([64, 64], f32, tag="RSH")
        RSW = eap.tile([64, 64], f32, tag="RSW")
        Ssm = eap.tile([64, 64], f32, tag="Ssm")
        RIh = eap.tile([64, 64], f32, tag="RIh")
        RIw = eap.tile([64, 64], f32, tag="RIw")
        vtp = es.enter_context(tc.tile_pool(name="vtp", bufs=8))
        psV = es.enter_context(tc.tile_pool(name="psV", bufs=2, space="PSUM"))

        def vt_w_chunk(wc):
            VT = vtp.tile([64, 4, 512], bf16, tag="VT")
            for i in range(4):
                w = wc * 4 + i
                ps = psV.tile([64, 512], f32, tag="vps")
                for cb in range(4):
                    nc.tensor.matmul(ps[:], _rows(src_in[cb])[:, 1:65, w],
                                     C['wv'][cb][:],
                                     start=(cb == 0), stop=(cb == 3))
                (nc.scalar.activation if i % 2 else nc.vector.tensor_copy)(
                    *((VT[:, i, :], ps[:], AF.Copy) if i % 2
                      else (VT[:, i, :], ps[:])))
            return VT

        def vt_h_chunk(hc):
            VT = vtp.tile([64, 4, 512], bf16, tag="VT")
            for i in range(4):
                h = hc * 4 + i
                ps = psV.tile([64, 512], f32, tag="vps")
                for cb in range(4):
                    nc.tensor.matmul(ps[:], _rows(src_in[cb])[:, h + 1, 0:64],
                                     C['wv'][cb][:],
                                     start=(cb == 0), stop=(cb == 3))
                (nc.scalar.activation if i % 2 else nc.vector.tensor_copy)(
                    *((VT[:, i, :], ps[:], AF.Copy) if i % 2
                      else (VT[:, i, :], ps[:])))
            return VT

        # ---- phase A: q/k convs + energies
        with contextlib.ExitStack() as esA:
            qkp = esA.enter_context(tc.tile_pool(name="qkp", bufs=1))
            psQ = esA.enter_context(tc.tile_pool(name="psQ", bufs=2, space="PSUM"))
            psE = esA.enter_context(tc.tile_pool(name="psE", bufs=4, space="PSUM"))
            q_sb = qkp.tile([64, 64, 65], bf16, tag="q")
            k_sb = qkp.tile([64, 64, 65], bf16, tag="k")
            for dst_sb, wgt, bias in [(q_sb, C['wq'], C['bq']),
                                      (k_sb, C['wk'], C['bk'])]:
                dflat = dst_sb[:].rearrange("p r c -> p (r c)")
                for off, n in QK_TILES:
                    ps = psQ.tile([64, 512], f32, tag="qkps")
                    for cb in range(4):
                        rhs = src_in[cb][:, IMG0 + off:IMG0 + off + n]
                        nc.tensor.matmul(ps[:, 0:n], wgt[cb][:], rhs,
                                         start=(cb == 0), stop=(cb == 3))
                    nc.scalar.activation(dflat[:, off:off + n], ps[:, 0:n],
                                         AF.Identity, bias=bias[:], scale=1.0)
            for wi in range(16):
                ps = psE.tile([64, 4, 64], f32, tag="e4")
                for k in range(4):
                    w = wi * 4 + k
                    nc.tensor.matmul(ps[:, k, :], q_sb[:, :, w], k_sb[:, :, w],
                                     start=True, stop=True)
                nc.vector.tensor_add(EH[:, wi * 4:wi * 4 + 4, :], ps[:],
                                     C['mask4'][:])
            for hi in range(16):
                ps = psE.tile([64, 4, 64], f32, tag="e4")
                for k in range(4):
                    h = hi * 4 + k
                    nc.tensor.matmul(ps[:, k, :], q_sb[:, h, 0:64],
                                     k_sb[:, h, 0:64], start=True, stop=True)
                nc.vector.tensor_copy(EW[:, hi * 4:hi * 4 + 4, :], ps[:])

        # ---- early VT (w-orientation) chunks 0..7: keeps PE busy in softmax
        vt_cache = {wc: vt_w_chunk(wc) for wc in range(8)}

        # ---- softmax (batched) + transposes
        with contextlib.ExitStack() as esS:
            ebp = esS.enter_context(tc.tile_pool(name="ebp", bufs=2))
            psS = esS.enter_context(tc.tile_pool(name="psS", bufs=2, space="PSUM"))
            psT = esS.enter_context(tc.tile_pool(name="psT", bufs=2, space="PSUM"))
            ehf = EH[:].rearrange("p a b -> p (a b)")
            ewf = EW[:].rearrange("p a b -> p (a b)")
            nc.scalar.activation(ehf, ehf, AF.Exp)
            nc.scalar.activation(ewf, ewf, AF.Exp)
            nc.vector.reduce_sum(RSH[:], EH[:], axis=AX.X)
            nc.vector.reduce_sum(RSW[:], EW[:], axis=AX.X)
            pst = psS.tile([64, 64], f32, tag="trS")
            nc.tensor.transpose(pst[:], RSW[:], identf[:])
            nc.vector.tensor_add(Ssm[:], RSH[:], pst[:])
            nc.vector.reciprocal(RIh[:], Ssm[:])
            pst2 = psS.tile([64, 64], f32, tag="trS")
            nc.tensor.transpose(pst2[:], Ssm[:], identf[:])
            nc.vector.reciprocal(RIw[:], pst2[:])
            for wi in range(16):
                eb = ebp.tile([64, 4, 64], bf16, tag="eb")
                pt = psT.tile([64, 4, 64], bf16, tag="at")
                for k in range(4):
                    w = wi * 4 + k
                    nc.scalar.activation(eb[:, k, :], EH[:, w, :], AF.Copy,
                                         scale=RIh[:, w:w + 1])
                    nc.tensor.transpose(pt[:, k, :], eb[:, k, :], ident[:])
                nc.scalar.activation(ATh[:, wi * 4:wi * 4 + 4, :], pt[:], AF.Copy)
            for hi in range(16):
                eb = ebp.tile([64, 4, 64], bf16, tag="eb")
                pt = psT.tile([64, 4, 64], bf16, tag="at")
                for k in range(4):
                    h = hi * 4 + k
                    nc.scalar.activation(eb[:, k, :], EW[:, h, :], AF.Copy,
                                         scale=RIw[:, h:h + 1])
                    nc.tensor.transpose(pt[:, k, :], eb[:, k, :], ident[:])
                nc.scalar.activation(ATw[:, hi * 4:hi * 4 + 4, :], pt[:], AF.Copy)

        # ---- phase C
        with contextlib.ExitStack() as esC:
            sgp = esC.enter_context(tc.tile_pool(name="sgp", bufs=3))
            psD = esC.enter_context(tc.tile_pool(name="psD", bufs=6, space="PSUM"))
            # w-phase: src_out = src_in + gamma*out_h
            for wc in range(16):
                VT = vt_cache.pop(wc) if wc in vt_cache else vt_w_chunk(wc)
                for cb in range(4):
                    pso = psD.tile([128, 4, 64], f32, tag="ops")
                    for i in range(4):
                        w = wc * 4 + i
                        nc.tensor.matmul(
                            pso[:, i, :], VT[:, i, cb * 128:(cb + 1) * 128],
                            ATh[:, w, :], start=True, stop=True)
                    stg = sgp.tile([128, 4, 64], bf16, tag="stg")
                    nc.scalar.activation(stg[:], pso[:], AF.Copy, scale=gamma)
                    o_sl = _rows(src_out[cb])[:, 1:65, wc * 4:wc * 4 + 4]
                    i_sl = _rows(src_in[cb])[:, 1:65, wc * 4:wc * 4 + 4]
                    nc.vector.tensor_add(o_sl, i_sl,
                                         stg[:].rearrange("p w h -> p h w"))
            # h-phase: src_out += gamma*out_w + gamma*v_b
            for hc in range(16):
                VT = vt_h_chunk(hc)
                for cb in range(4):
                    pso = psD.tile([128, 4, 64], f32, tag="ops")
                    for i in range(4):
                        h = hc * 4 + i
                        nc.tensor.matmul(
                            pso[:, i, :], VT[:, i, cb * 128:(cb + 1) * 128],
                            ATw[:, h, :], start=True, stop=True)
                    stg = sgp.tile([128, 4, 64], bf16, tag="stg")
                    nc.scalar.activation(stg[:], pso[:], AF.Identity,
                                         scale=gamma, bias=C['gvb'][cb][:])
                    o_sl = _rows(src_out[cb])[:, 1 + hc * 4:1 + hc * 4 + 4, 0:64]
                    nc.vector.tensor_add(o_sl, o_sl, stg[:])


_BUILD_CACHE = {}


def _get_nc(gamma):
    key = round(float(gamma), 12)
    if key not in _BUILD_CACHE:
        _BUILD_CACHE[key] = build(gamma, n_reps=1)
    return _BUILD_CACHE[key]


def kernel(**inputs):
    from concourse.bass_utils import run_bass_kernel_spmd
    inputs_np = {k: np.asarray(v) for k, v in inputs.items()}
    dev, gamma = host_prep(inputs_np)
    nc = _get_nc(gamma)
    in_maps = []
    for core in range(8):
        m = dict(dev)
        m['x'] = pad_x_host(np.asarray(inputs_np['x'][core], np.float32))
        in_maps.append(m)
    res = run_bass_kernel_spmd(nc, in_maps, core_ids=list(range(8)))
    out = np.stack([r['y'].reshape(104, 64, 64) for r in res.results])
    return out.astype(np.float32)
